# revision 3
# baseline (speedup 1.0000x reference)
"""Embedding lookup kernel for Trainium2 (8 NeuronCores, SPMD).

Strategy: token-parallel gather. The reference one-hot+matmul is just an
embedding row gather: out[b, s, :] = weight[x[b, s], :].

- Flatten x [2, 4096] -> [8192] tokens; each of the 8 cores handles 1024
  contiguous tokens.
- Each core receives the FULL weight table [32000, 128] f32 in its DRAM and
  its 1024 indices laid out as [128, 8] int32 (partition p holds tokens
  p*8 .. p*8+7).
- 8 indirect DMAs (SWDGE gather), one per token column j: index tile column
  [128, 1] -> dest [128, 128] f32 slice, so partition p cols j*128..(j+1)*128
  = weight[x[p*8+j]].  (HW consumes exactly one index per dest partition row
  per gather op, so the multi-index-per-partition form is not usable.)
- One direct DMA writes the tile to the output DRAM tensor [128, 1024],
  which reshapes to [1024, 128] row-major == tokens in order.

No collectives needed; the host concatenates the 8 per-core outputs.
"""

import numpy as np

import concourse.bass as bass
import concourse.tile as tile
from concourse import bacc, mybir
from concourse.bass_utils import run_bass_kernel_spmd

N_CORES = 8
B, S = 2, 4096
VOCAB, DIM = 32000, 128
P = 128
TOKENS = B * S                      # 8192
TPC = TOKENS // N_CORES             # 1024 tokens per core
TPP = TPC // P                      # 8 tokens per partition


def build_nc():
    nc = bacc.Bacc(None, target_bir_lowering=False)
    x = nc.dram_tensor("x", [P, TPP], mybir.dt.int32, kind="ExternalInput")
    w = nc.dram_tensor("weight", [VOCAB, DIM], mybir.dt.float32, kind="ExternalInput")
    out = nc.dram_tensor("out", [P, TPC], mybir.dt.float32, kind="ExternalOutput")

    with tile.TileContext(nc) as tc:
        with tc.tile_pool(name="sbuf", bufs=1) as pool:
            idx_tile = pool.tile([P, TPP], mybir.dt.int32)
            nc.sync.dma_start(out=idx_tile[:], in_=x[:])
            g = pool.tile([P, TPC], mybir.dt.float32)
            for j in range(TPP):
                nc.gpsimd.indirect_dma_start(
                    out=g[:, j * DIM : (j + 1) * DIM],
                    out_offset=None,
                    in_=w[:],
                    in_offset=bass.IndirectOffsetOnAxis(
                        ap=idx_tile[:, j : j + 1], axis=0
                    ),
                )
            nc.sync.dma_start(out=out[:], in_=g[:])
    nc.compile()
    return nc


_NC_CACHE = None


def kernel(x: np.ndarray, weight: np.ndarray, **run_kwargs):
    global _NC_CACHE
    if _NC_CACHE is None:
        _NC_CACHE = build_nc()
    nc = _NC_CACHE

    x_flat = np.ascontiguousarray(np.asarray(x).reshape(-1).astype(np.int32))
    w = np.ascontiguousarray(np.asarray(weight, dtype=np.float32))

    in_maps = [
        {
            "x": x_flat[c * TPC : (c + 1) * TPC].reshape(P, TPP),
            "weight": w,
        }
        for c in range(N_CORES)
    ]
    res = run_bass_kernel_spmd(nc, in_maps, core_ids=list(range(N_CORES)), **run_kwargs)
    parts = [res.results[c]["out"].reshape(TPC, DIM) for c in range(N_CORES)]
    full = np.concatenate(parts, axis=0).reshape(B, S, DIM)
    if run_kwargs:
        return full, res
    return full
